# revision 17
# baseline (speedup 1.0000x reference)
"""CylinderGRUDecoder Trainium2 kernel (8-core SPMD, bass/Tile) — v2.

Strategy (v2, beyond the v1 baseline)
-------------------------------------
Same outer decomposition as v1: coords are randint(0, 32), so only the
32^3 corner of each grid is touched; the h0 gather happens host-side and
each of the 8 cores gets 25000 points of one batch as 25 pairs of
512-point tiles (pair halves A/B live on SBUF partitions 0-63 / 64-127,
so elementwise work always runs 128 lanes wide).

v2 rebalances the per-iteration GRU work across ALL engines:
  PE   : r/z gates via bf16 block-diagonal matmuls; the q gate runs as a
         single fp8e4m3 DoubleRow matmul (lhsT [128,2,128] = [Wq_h|Wq_x],
         rhs [128,2,512] = [r*h | x] stacked along the free dim) -- half
         the cycles/row and one matmul instead of two.
  ACT  : exact sigmoid(z) and tanh(q) only, batched two pairs per
         instruction ([128,1024]); gelu in the decoder tail.
  DVE  : a custom fused DVE op (registered at import into the per-NEFF
         DVE table) computes r*h = (0.5 + t*(c0+c1*t^2))*h with
         t = clamp(s_r, +-1) in ONE pass -- the deg-3 odd-poly sigmoid
         (max err 1.9e-2, s_r pre-scaled by 1/3.75 folded into Wr; |s_r|
         stays < 3.2 so the clamp never truncates) -- writing fp8
         directly for the q matmul. r-gate errors are strongly damped
         through Wq: end-to-end effect measured at ~+6e-4 abs.
  Pool : the blend's elementwise adds/muls that don't fit on DVE run as
         GPSIMD scalar_tensor_tensor ops.
The blend is the numerically robust D-form: h' = h + z*(q - h).
"""

import os
import sys

import numpy as np

try:
    import concourse.bass as bass  # noqa: F401
except Exception:  # pragma: no cover
    for _p in ("/opt/trn_rl_repo", "/root/.axon_site/_ro/trn_rl_repo"):
        if os.path.isdir(_p) and _p not in sys.path:
            sys.path.insert(0, _p)

import concourse.bass as bass
import concourse.tile as tile
from concourse import bacc, mybir
from concourse.bass_utils import run_bass_kernel_spmd

import ml_dtypes

BF16 = ml_dtypes.bfloat16
F8E4 = ml_dtypes.float8_e4m3

# problem constants (hardcoded per harness contract)
B = 2
N = 100000
C_HALF = 32
HID = 64
PFEAT = 64
NUM_ITERS = 4
GRID_SIDE = 32                      # coords in [0, 32)
NCELL = GRID_SIDE ** 3              # 32768 rows
NCORES = 8
NP_CORE = N // 4                    # 25000 real points per core
F = 512                             # point-tile free dim
NPAIR = 25                          # pairs per core
NP_PAD = NPAIR * 2 * F              # 25600 padded points per core
NCOL = NPAIR * F                    # 12800 free columns of state

# sigmoid(x) ~ 0.5 + t*(SIG_C0 + SIG_C1*t^2), t = clamp(x/SIG_K, -1, 1)
# (Lawson-minimax deg-3 odd fit; in-range max err 1.9e-2, |s_r| <= 3.2)
SIG_K = 3.75
SIG_C0 = 0.82063637
SIG_C1 = -0.36275319

FP8_Q = True     # q-gate matmul via fp8e4m3 DoubleRow
_CACHED = {}
GELU_FUNC = mybir.ActivationFunctionType.Gelu
REPEATS = 1  # >1 only for wall-clock timing experiments

# flexible-op placement: which of {d, e, h} go to GPSIMD (Pool) per pair
# (real TRN2 GPSIMD only runs plain TensorTensor ops, at 0.42 efficiency)
def _pool_d(g):
    return False

def _pool_e(g):
    return False

def _pool_h(g):
    return True


def _register_sigmul():
    """Register the fused clamp+poly-sigmoid-multiply custom DVE op."""
    import concourse.dve_ops as dve_ops
    name = "ANT_SIGMUL_V1"
    for op in dve_ops.OPS:
        if op.name == name:
            return op
    from concourse.dve_spec import (
        Spec, Src0, Src1, C0, C1, C2, Zero, One, minn, maxx, lower, _has_src1,
    )
    from concourse.dve_uop import DveOpSpec

    def _ref(in0, in1, c0, c1, c2):
        t = np.clip(in0.astype(np.float32), -1.0, 1.0)
        return ((c2 + t * (c0 + c1 * t * t)) * in1.astype(np.float32)
                ).astype(np.float32)

    t = maxx(minn(Src0, One), Zero - One)
    body = (C2 + t * (C0 + C1 * (t * t))) * Src1
    spec = Spec(body=body, reference=_ref)
    row = max(dve_ops._SUB_OPCODE_FOR_NAME.values()) + 1
    assert row < 0x20, "custom-DVE opcode rows exhausted"
    uops = lower(spec, ver="v3")
    sha = DveOpSpec(name=name, opcode=row, uops=uops,
                    rd1_en=_has_src1(spec)).sha("v3")
    op = dve_ops.DveOp(name, spec, subdim=False, uops_sha={"v3": sha})
    dve_ops._SUB_OPCODE_FOR_NAME[name] = row
    dve_ops.OPS.append(op)
    dve_ops.CUSTOM_DVE_SPECS[name] = spec
    return op


SIGMUL = _register_sigmul()

# weight column offsets inside wts (block-diagonal lhsT layouts: the
# A half [rows 0-63] feeds output cols 0-63, B [rows 64-127] cols 64-127)
WRH, WZH, WQH = 0, 128, 256
WRX, WZX, WQX = 384, 512, 640
WD1H, WD1X, WD2Q = 768, 832, 896
WCOLS = 912

# pair groups: twelve 2-pair groups + one single
GROUPS = [(2 * i, 2 * i + 1) for i in range(NPAIR // 2)] + [(NPAIR - 1,)]


def _build_program():
    """Build the SPMD Bass program (identical on all 8 cores)."""
    nc = bacc.Bacc(trn_type="TRN2", target_bir_lowering=False, debug=False,
                   enable_asserts=True, num_devices=NCORES)
    dt = mybir.dt

    h0p_d = nc.dram_tensor("h0p", [128, NCOL], dt.bfloat16,
                           kind="ExternalInput").ap()
    xp_d = nc.dram_tensor("xp", [128, NCOL], dt.bfloat16,
                          kind="ExternalInput").ap()
    xp8_d = nc.dram_tensor("xp8", [128, NCOL], dt.float8e4,
                           kind="ExternalInput").ap()
    wts_d = nc.dram_tensor("wts", [128, WCOLS], dt.bfloat16,
                           kind="ExternalInput").ap()
    wq8_d = nc.dram_tensor("wq8", [128, 256], dt.float8e4,
                           kind="ExternalInput").ap()
    flow_d = nc.dram_tensor("flow", [8, NCOL], dt.float32,
                            kind="ExternalOutput").ap()

    with tile.TileContext(nc) as tc:
        with (
            tc.tile_pool(name="singles", bufs=1) as singles,
            tc.tile_pool(name="zpool", bufs=3) as zpool,
            tc.tile_pool(name="qtpool", bufs=3) as qtpool,
            tc.tile_pool(name="rhpool", bufs=3) as rhpool,
            tc.tile_pool(name="dpool", bufs=4) as dpool,
            tc.tile_pool(name="epool", bufs=4) as epool,
            tc.tile_pool(name="hmpool", bufs=2) as hmpool,
            tc.tile_pool(name="flpool", bufs=2) as flpool,
            tc.tile_pool(name="prz", bufs=2, space="PSUM") as przp,
        ):
            wts = singles.tile([128, WCOLS], dt.bfloat16)
            nc.sync.dma_start(out=wts, in_=wts_d[:])
            if FP8_Q:
                wq8 = singles.tile([128, 256], dt.float8e4)
                nc.sync.dma_start(out=wq8, in_=wq8_d[:])

            # all pairs' state resident: G = [h_A; h_B], X = [x_A; x_B]
            Gb = singles.tile([128, NCOL], dt.bfloat16)
            Xb = singles.tile([128, NCOL], dt.bfloat16)
            if FP8_Q:
                # [r*h | x] fp8 shadow, stacked along free for DoubleRow
                RX8 = singles.tile([128, 2 * NCOL], dt.float8e4)

            # stage loads: first two pairs alone so slot 0 starts early
            for c0, c1 in ([(0, 2), (2, 4)] +
                           [(c, min(c + 7, NPAIR)) for c in range(4, NPAIR, 7)]):
                s = slice(c0 * F, c1 * F)
                nc.sync.dma_start(out=Gb[:, s], in_=h0p_d[:, s])
                nc.sync.dma_start(out=Xb[:, s], in_=xp_d[:, s])
                if FP8_Q:
                    nc.sync.dma_start(out=RX8[:, NCOL + c0 * F:
                                              NCOL + c1 * F],
                                      in_=xp8_d[:, s])

            def acc_mm(psum_out, wh_col, wx_col, rhs_h, rhs_x, m):
                nc.tensor.matmul(out=psum_out,
                                 lhsT=wts[:, wh_col:wh_col + m],
                                 rhs=rhs_h, start=True, stop=False)
                nc.tensor.matmul(out=psum_out,
                                 lhsT=wts[:, wx_col:wx_col + m],
                                 rhs=rhs_x, start=False, stop=True)

            if FP8_Q:
                rx8_3d = RX8[:, :].rearrange("p (two n) -> p two n", two=2)
                wq8_3d = wq8[:, :].rearrange("p (two m) -> p two m", two=2)

            # --- software-pipelined GRU ---------------------------------
            # slot s: head(A=s): rz matmuls + fused r*h (custom DVE)
            #         mid (B=s-1): q matmul (into A's dead r-half of prz),
            #                      sigmoid(z), tanh(q)
            #         tail(C=s-2): blend d/e/h' on DVE+Pool
            # This keeps every in-order engine queue stall-free.
            state = {}  # slot -> dict(prz=, Z=, QT=, grp=)

            def emit_head(s):
                it, grp = slots[s]
                g0 = grp[0]
                W = len(grp) * F
                cols = slice(g0 * F, g0 * F + W)
                prz = przp.tile([128, 4 * F], dt.float32, tag="a")
                for j, g in enumerate(grp):      # r matmuls first
                    gc = slice(g * F, (g + 1) * F)
                    acc_mm(prz[:, j * F:(j + 1) * F],
                           WRH, WRX, Gb[:, gc], Xb[:, gc], 128)
                for j, g in enumerate(grp):
                    gc = slice(g * F, (g + 1) * F)
                    acc_mm(prz[:, 2 * F + j * F:2 * F + (j + 1) * F],
                           WZH, WZX, Gb[:, gc], Xb[:, gc], 128)
                if FP8_Q:
                    rh_out = RX8[:, cols]
                else:
                    rh_t = rhpool.tile([128, 2 * F], dt.bfloat16)
                    rh_out = rh_t[:, 0:W]
                nc.vector._custom_dve(
                    SIGMUL, out=rh_out, in0=prz[:, 0:W],
                    in1=Gb[:, cols], s0=SIG_C0, s1=SIG_C1, imm2=0.5)
                state[s] = dict(grp=grp, prz=prz, rh=rh_out, W=W)

            def emit_mid(s):
                st = state[s]
                grp, prz, W = st["grp"], st["prz"], st["W"]
                for j, g in enumerate(grp):
                    if FP8_Q:
                        nc.tensor.matmul(
                            out=prz[:, j * F:(j + 1) * F],
                            lhsT=wq8_3d,
                            rhs=rx8_3d[:, :, g * F:(g + 1) * F],
                            start=True, stop=True,
                            perf_mode=mybir.MatmulPerfMode.DoubleRow)
                    else:
                        acc_mm(prz[:, j * F:(j + 1) * F], WQH, WQX,
                               st["rh"][:, j * F:(j + 1) * F],
                               Xb[:, g * F:(g + 1) * F], 128)
                QT = qtpool.tile([128, 2 * F], dt.bfloat16)
                nc.scalar.activation(
                    out=QT[:, 0:W], in_=prz[:, 0:W],
                    func=mybir.ActivationFunctionType.Tanh)
                Z = zpool.tile([128, 2 * F], dt.bfloat16)
                nc.scalar.activation(
                    out=Z[:, 0:W], in_=prz[:, 2 * F:2 * F + W],
                    func=mybir.ActivationFunctionType.Sigmoid)
                st["Z"], st["QT"] = Z, QT

            def emit_de(s):
                st = state[s]
                grp, Z, QT = st["grp"], st["Z"], st["QT"]
                st["E"] = []
                for j, g in enumerate(grp):
                    gc = slice(g * F, (g + 1) * F)
                    jc = slice(j * F, (j + 1) * F)
                    D = dpool.tile([128, F], dt.bfloat16)
                    if _pool_d(g):
                        nc.gpsimd.tensor_tensor(
                            out=D, in0=QT[:, jc], in1=Gb[:, gc],
                            op=mybir.AluOpType.subtract)
                    else:
                        nc.vector.tensor_tensor(
                            out=D, in0=QT[:, jc], in1=Gb[:, gc],
                            op=mybir.AluOpType.subtract)
                    E = epool.tile([128, F], dt.bfloat16)
                    if _pool_e(g):
                        nc.gpsimd.tensor_mul(E, Z[:, jc], D)
                    else:
                        nc.vector.tensor_mul(E, Z[:, jc], D)
                    st["E"].append(E)

            def emit_hupd(s):
                st = state.pop(s)
                grp = st["grp"]
                for j, g in enumerate(grp):
                    gc = slice(g * F, (g + 1) * F)
                    E = st["E"][j]
                    if _pool_h(g):
                        nc.gpsimd.tensor_add(Gb[:, gc], Gb[:, gc], E)
                    else:
                        nc.vector.tensor_add(Gb[:, gc], Gb[:, gc], E)

            for _ in range(REPEATS):
                slots = [(it, grp) for it in range(NUM_ITERS)
                         for grp in GROUPS]
                for s in range(len(slots) + 3):
                    if s >= 3:
                        emit_hupd(s - 3)
                    if s >= 2 and s - 2 < len(slots):
                        emit_de(s - 2)
                    if s >= 1 and s - 1 < len(slots):
                        emit_mid(s - 1)
                    if s < len(slots):
                        emit_head(s)

                # decoder (pipelined, 2 groups = 4 pairs per round):
                # hmid = gelu(Wd1 @ [h; x]), flow = Wd2 @ hmid
                ROUNDS = [GROUPS[i:i + 2] for i in range(0, len(GROUPS), 2)]
                dstate = {}

                def dec_head(di):
                    pd = przp.tile([128, 2 * F], dt.float32, tag="a")
                    pairs = []
                    for hi, grp in enumerate(ROUNDS[di]):
                        for j, g in enumerate(grp):
                            gc = slice(g * F, (g + 1) * F)
                            acc_mm(pd[64 * j:64 * (j + 1),
                                      hi * F:(hi + 1) * F], WD1H, WD1X,
                                   Gb[:, gc], Xb[:, gc], 64)
                        pairs.append(grp)
                    dstate[di] = (pd, pairs)

                def dec_mid(di):
                    pd, pairs = dstate[di]
                    hm = hmpool.tile([128, 2 * F], dt.bfloat16)
                    for hi, grp in enumerate(pairs):
                        n = len(grp)
                        nc.scalar.activation(
                            out=hm[0:64 * n, hi * F:(hi + 1) * F],
                            in_=pd[0:64 * n, hi * F:(hi + 1) * F],
                            func=GELU_FUNC)
                    dstate[di] = (pd, pairs, hm)

                def dec_tail(di):
                    pd, pairs, hm = dstate.pop(di)
                    fl = flpool.tile([16, 2 * F], dt.float32)
                    for hi, grp in enumerate(pairs):
                        n = len(grp)
                        pf = pd[0:16, hi * F:(hi + 1) * F]
                        nc.tensor.matmul(
                            out=pf[0:8 * n, :],
                            lhsT=wts[0:64 * n, WD2Q:WD2Q + 8 * n],
                            rhs=hm[0:64 * n, hi * F:(hi + 1) * F],
                            start=True, stop=True)
                        nc.vector.tensor_copy(
                            out=fl[0:8 * n, hi * F:(hi + 1) * F],
                            in_=pf[0:8 * n, :])
                        for j, g in enumerate(grp):
                            nc.sync.dma_start(
                                out=flow_d[:, g * F:(g + 1) * F],
                                in_=fl[8 * j:8 * j + 8,
                                       hi * F:(hi + 1) * F])

                for di in range(len(ROUNDS) + 2):
                    if di >= 2:
                        dec_tail(di - 2)
                    if di >= 1 and di - 1 < len(ROUNDS):
                        dec_mid(di - 1)
                    if di < len(ROUNDS):
                        dec_head(di)

    nc.finalize()
    return nc


def _prep_host(before_feats, after_feats, point_feats, coords):
    """Build per-core input maps."""
    bf = np.asarray(before_feats)
    af = np.asarray(after_feats)
    pf = np.asarray(point_feats)
    cd = np.asarray(coords)
    assert cd.max() < GRID_SIDE and cd.min() >= 0, "coords out of 32^3 corner"

    # per-batch grid corner: [64, NCELL] f32
    grids = []
    for b in range(B):
        sub_b = bf[b, :, :GRID_SIDE, :GRID_SIDE, :GRID_SIDE]
        sub_a = af[b, :, :GRID_SIDE, :GRID_SIDE, :GRID_SIDE]
        grids.append(np.concatenate([sub_b, sub_a], axis=0)
                     .reshape(HID, NCELL))

    flat = ((cd[..., 0].astype(np.int64) * GRID_SIDE + cd[..., 1])
            * GRID_SIDE + cd[..., 2])               # [B, N]

    in_maps = []
    for core in range(NCORES):
        b, q = divmod(core, 4)
        sl = slice(q * NP_CORE, (q + 1) * NP_CORE)

        h0 = np.zeros((HID, NP_PAD), dtype=BF16)
        h0[:, :NP_CORE] = grids[b].take(flat[b, sl], axis=1).astype(BF16)
        # h0p[0:64] = h of A-halves (first 512 of each 1024), h0p[64:128] = B
        h0p = np.empty((128, NCOL), dtype=BF16)
        h03 = h0.reshape(HID, NPAIR, 2 * F)
        h0p[0:64] = h03[:, :, :F].reshape(HID, NCOL)
        h0p[64:128] = h03[:, :, F:].reshape(HID, NCOL)

        xt = np.zeros((PFEAT, NP_PAD), dtype=BF16)
        xt[:, :NP_CORE] = pf[b, sl].T.astype(BF16)
        xp = np.empty((128, NCOL), dtype=BF16)
        xt3 = xt.reshape(PFEAT, NPAIR, 2 * F)
        xp[0:64] = xt3[:, :, :F].reshape(PFEAT, NCOL)
        xp[64:128] = xt3[:, :, F:].reshape(PFEAT, NCOL)

        m = {
            "h0p": np.ascontiguousarray(h0p),
            "xp": np.ascontiguousarray(xp),
            "wts": _CACHED["wts"],
        }
        if FP8_Q:
            m["xp8"] = np.ascontiguousarray(xp.astype(F8E4))
            m["wq8"] = _CACHED["wq8"]
        in_maps.append(m)
    return in_maps


def _pack_weights(Wz, Wr, Wq, Wd1, Wd2):
    """Block-diagonal lhsT layouts: rows 0-63 (A state) feed the first
    half of the output columns, rows 64-127 (B) the second half."""
    w = np.zeros((128, WCOLS), dtype=BF16)
    Wzb = np.asarray(Wz).astype(np.float32)
    Wrb = np.asarray(Wr).astype(np.float32) / SIG_K   # pre-scale for SIGMUL
    Wqb = np.asarray(Wq).astype(np.float32)
    Wd1b = np.asarray(Wd1).astype(BF16)
    Wd2b = np.asarray(Wd2).astype(BF16)

    def blockdiag(col, wt):  # wt: lhsT block [64, m]
        m = wt.shape[1]
        w[0:64, col:col + m] = wt
        w[64:128, col + m:col + 2 * m] = wt

    blockdiag(WRH, Wrb[:, :HID].T.astype(BF16))
    blockdiag(WZH, Wzb[:, :HID].T.astype(BF16))
    blockdiag(WQH, Wqb[:, :HID].T.astype(BF16))
    blockdiag(WRX, Wrb[:, HID:].T.astype(BF16))
    blockdiag(WZX, Wzb[:, HID:].T.astype(BF16))
    blockdiag(WQX, Wqb[:, HID:].T.astype(BF16))
    blockdiag(WD1H, Wd1b[:, :HID].T)
    blockdiag(WD1X, Wd1b[:, HID:].T)
    # WD2Q: [128, 16]: 4 x 32-row blocks -> 4-col blocks (A/B of 2 pairs)
    for blk in range(4):
        w[32 * blk:32 * (blk + 1), WD2Q + 4 * blk:WD2Q + 4 * blk + 3] = \
            Wd2b.T

    # fp8 q weights, stacked for DoubleRow: cols 0:128 = Wq_h, 128:256 = Wq_x
    wq8 = np.zeros((128, 256), dtype=F8E4)
    wq8[0:64, 0:64] = Wqb[:, :HID].T.astype(F8E4)
    wq8[64:128, 64:128] = Wqb[:, :HID].T.astype(F8E4)
    wq8[0:64, 128:192] = Wqb[:, HID:].T.astype(F8E4)
    wq8[64:128, 192:256] = Wqb[:, HID:].T.astype(F8E4)
    return np.ascontiguousarray(w), np.ascontiguousarray(wq8)


def kernel(before_feats, after_feats, point_feats, coords,
           Wz, bz, Wr, br, Wq, bq, Wd1, bd1, Wd2, bd2):
    for bias in (bz, br, bq, bd1):
        assert np.abs(np.asarray(bias)).max() == 0.0, "nonzero bias unsupported"

    if "nc" not in _CACHED:
        _CACHED["nc"] = _build_program()
    _CACHED["wts"], _CACHED["wq8"] = _pack_weights(Wz, Wr, Wq, Wd1, Wd2)

    in_maps = _prep_host(before_feats, after_feats, point_feats, coords)
    res = run_bass_kernel_spmd(_CACHED["nc"], in_maps, list(range(NCORES)))
    _CACHED["last_exec_time_ns"] = res.exec_time_ns
    _CACHED["last_mean_exec_time_ns"] = res.mean_exec_time_ns

    out = np.empty((B, N, 3), dtype=np.float32)
    bd2v = np.asarray(bd2).astype(np.float32).reshape(1, 3)
    for core in range(NCORES):
        b, q = divmod(core, 4)
        fl = res.results[core]["flow"]          # [8, NCOL]
        fl3 = fl.reshape(8, NPAIR, F)
        per_pt = np.empty((3, NP_PAD), dtype=np.float32)
        pp = per_pt.reshape(3, NPAIR, 2 * F)
        pp[:, :, :F] = fl3[0:3]
        pp[:, :, F:] = fl3[4:7]
        out[b, q * NP_CORE:(q + 1) * NP_CORE, :] = \
            per_pt[:, :NP_CORE].T + bd2v
    return out


# revision 18
# speedup vs baseline: 1.0261x; 1.0261x over previous
"""CylinderGRUDecoder Trainium2 kernel (8-core SPMD, bass/Tile) — v2.

Strategy (v2, beyond the v1 baseline)
-------------------------------------
Same outer decomposition as v1: coords are randint(0, 32), so only the
32^3 corner of each grid is touched; the h0 gather happens host-side and
each of the 8 cores gets 25000 points of one batch as 25 pairs of
512-point tiles (pair halves A/B live on SBUF partitions 0-63 / 64-127,
so elementwise work always runs 128 lanes wide).

v2 rebalances the per-iteration GRU work across ALL engines:
  PE   : r/z gates via bf16 block-diagonal matmuls; the q gate runs as a
         single fp8e4m3 DoubleRow matmul (lhsT [128,2,128] = [Wq_h|Wq_x],
         rhs [128,2,512] = [r*h | x] stacked along the free dim) -- half
         the cycles/row and one matmul instead of two.
  ACT  : exact sigmoid(z) and tanh(q) only, batched two pairs per
         instruction ([128,1024]); gelu in the decoder tail.
  DVE  : a custom fused DVE op (registered at import into the per-NEFF
         DVE table) computes r*h = (0.5 + t*(c0+c1*t^2))*h with
         t = clamp(s_r, +-1) in ONE pass -- the deg-3 odd-poly sigmoid
         (max err 1.9e-2, s_r pre-scaled by 1/3.75 folded into Wr; |s_r|
         stays < 3.2 so the clamp never truncates) -- writing fp8
         directly for the q matmul. r-gate errors are strongly damped
         through Wq: end-to-end effect measured at ~+6e-4 abs.
  Pool : the blend's elementwise adds/muls that don't fit on DVE run as
         GPSIMD scalar_tensor_tensor ops.
The blend is the numerically robust D-form: h' = h + z*(q - h).
"""

import os
import sys

import numpy as np

try:
    import concourse.bass as bass  # noqa: F401
except Exception:  # pragma: no cover
    for _p in ("/opt/trn_rl_repo", "/root/.axon_site/_ro/trn_rl_repo"):
        if os.path.isdir(_p) and _p not in sys.path:
            sys.path.insert(0, _p)

import concourse.bass as bass
import concourse.tile as tile
from concourse import bacc, mybir
from concourse.bass_utils import run_bass_kernel_spmd

import ml_dtypes

BF16 = ml_dtypes.bfloat16
F8E4 = ml_dtypes.float8_e4m3

# problem constants (hardcoded per harness contract)
B = 2
N = 100000
C_HALF = 32
HID = 64
PFEAT = 64
NUM_ITERS = 4
GRID_SIDE = 32                      # coords in [0, 32)
NCELL = GRID_SIDE ** 3              # 32768 rows
NCORES = 8
NP_CORE = N // 4                    # 25000 real points per core
F = 512                             # point-tile free dim
NPAIR = 25                          # pairs per core
NP_PAD = NPAIR * 2 * F              # 25600 padded points per core
NCOL = NPAIR * F                    # 12800 free columns of state

# sigmoid(x) ~ 0.5 + t*(SIG_C0 + SIG_C1*t^2), t = clamp(x/SIG_K, -1, 1)
# (Lawson-minimax deg-3 odd fit; in-range max err 1.9e-2, |s_r| <= 3.2)
SIG_K = 3.75
SIG_C0 = 0.82063637
SIG_C1 = -0.36275319

FP8_Q = True     # q-gate matmul via fp8e4m3 DoubleRow
_CACHED = {}
GELU_FUNC = mybir.ActivationFunctionType.Gelu
REPEATS = 1  # >1 only for wall-clock timing experiments

# flexible-op placement: which of {d, e, h} go to GPSIMD (Pool) per pair
# (real TRN2 GPSIMD only runs plain TensorTensor ops, at 0.42 efficiency)
def _pool_d(g):
    return False

def _pool_e(g):
    return False

def _pool_h(g):
    return True


def _register_sigmul():
    """Register the fused clamp+poly-sigmoid-multiply custom DVE op."""
    import concourse.dve_ops as dve_ops
    name = "ANT_SIGMUL_V1"
    for op in dve_ops.OPS:
        if op.name == name:
            return op
    from concourse.dve_spec import (
        Spec, Src0, Src1, C0, C1, C2, Zero, One, minn, maxx, lower, _has_src1,
    )
    from concourse.dve_uop import DveOpSpec

    def _ref(in0, in1, c0, c1, c2):
        t = np.clip(in0.astype(np.float32), -1.0, 1.0)
        return ((c2 + t * (c0 + c1 * t * t)) * in1.astype(np.float32)
                ).astype(np.float32)

    t = maxx(minn(Src0, One), Zero - One)
    body = (C2 + t * (C0 + C1 * (t * t))) * Src1
    spec = Spec(body=body, reference=_ref)
    row = max(dve_ops._SUB_OPCODE_FOR_NAME.values()) + 1
    assert row < 0x20, "custom-DVE opcode rows exhausted"
    uops = lower(spec, ver="v3")
    sha = DveOpSpec(name=name, opcode=row, uops=uops,
                    rd1_en=_has_src1(spec)).sha("v3")
    op = dve_ops.DveOp(name, spec, subdim=False, uops_sha={"v3": sha})
    dve_ops._SUB_OPCODE_FOR_NAME[name] = row
    dve_ops.OPS.append(op)
    dve_ops.CUSTOM_DVE_SPECS[name] = spec
    return op


SIGMUL = _register_sigmul()

# weight column offsets inside wts (block-diagonal lhsT layouts: the
# A half [rows 0-63] feeds output cols 0-63, B [rows 64-127] cols 64-127)
WRH, WZH, WQH = 0, 128, 256
WRX, WZX, WQX = 384, 512, 640
WD1H, WD1X, WD2Q = 768, 832, 896
WCOLS = 912

# pair groups: twelve 2-pair groups + one single
GROUPS = [(2 * i, 2 * i + 1) for i in range(NPAIR // 2)] + [(NPAIR - 1,)]


def _build_program():
    """Build the SPMD Bass program (identical on all 8 cores)."""
    nc = bacc.Bacc(trn_type="TRN2", target_bir_lowering=False, debug=False,
                   enable_asserts=True, num_devices=NCORES)
    dt = mybir.dt

    h0p_d = nc.dram_tensor("h0p", [128, NCOL], dt.bfloat16,
                           kind="ExternalInput").ap()
    xp_d = nc.dram_tensor("xp", [128, NCOL], dt.bfloat16,
                          kind="ExternalInput").ap()
    xp8_d = nc.dram_tensor("xp8", [128, NCOL], dt.float8e4,
                           kind="ExternalInput").ap()
    wts_d = nc.dram_tensor("wts", [128, WCOLS], dt.bfloat16,
                           kind="ExternalInput").ap()
    wq8_d = nc.dram_tensor("wq8", [128, 256], dt.float8e4,
                           kind="ExternalInput").ap()
    flow_d = nc.dram_tensor("flow", [8, NCOL], dt.float32,
                            kind="ExternalOutput").ap()

    with tile.TileContext(nc) as tc:
        with (
            tc.tile_pool(name="singles", bufs=1) as singles,
            tc.tile_pool(name="zpool", bufs=3) as zpool,
            tc.tile_pool(name="qtpool", bufs=3) as qtpool,
            tc.tile_pool(name="rhpool", bufs=3) as rhpool,
            tc.tile_pool(name="dpool", bufs=4) as dpool,
            tc.tile_pool(name="epool", bufs=4) as epool,
            tc.tile_pool(name="hmpool", bufs=2) as hmpool,
            tc.tile_pool(name="flpool", bufs=2) as flpool,
            tc.tile_pool(name="prz", bufs=2, space="PSUM") as przp,
        ):
            wts = singles.tile([128, WCOLS], dt.bfloat16)
            nc.sync.dma_start(out=wts, in_=wts_d[:])
            if FP8_Q:
                wq8 = singles.tile([128, 256], dt.float8e4)
                nc.sync.dma_start(out=wq8, in_=wq8_d[:])

            # all pairs' state resident: G = [h_A; h_B], X = [x_A; x_B]
            Gb = singles.tile([128, NCOL], dt.bfloat16)
            Xb = singles.tile([128, NCOL], dt.bfloat16)
            if FP8_Q:
                # [r*h | x] fp8 shadow, stacked along free for DoubleRow
                RX8 = singles.tile([128, 2 * NCOL], dt.float8e4)

            # stage loads: first two pairs alone so slot 0 starts early
            for c0, c1 in ([(0, 2), (2, 4)] +
                           [(c, min(c + 7, NPAIR)) for c in range(4, NPAIR, 7)]):
                s = slice(c0 * F, c1 * F)
                nc.sync.dma_start(out=Gb[:, s], in_=h0p_d[:, s])
                nc.sync.dma_start(out=Xb[:, s], in_=xp_d[:, s])
                if FP8_Q:
                    nc.sync.dma_start(out=RX8[:, NCOL + c0 * F:
                                              NCOL + c1 * F],
                                      in_=xp8_d[:, s])

            def acc_mm(psum_out, wh_col, wx_col, rhs_h, rhs_x, m):
                nc.tensor.matmul(out=psum_out,
                                 lhsT=wts[:, wh_col:wh_col + m],
                                 rhs=rhs_h, start=True, stop=False)
                nc.tensor.matmul(out=psum_out,
                                 lhsT=wts[:, wx_col:wx_col + m],
                                 rhs=rhs_x, start=False, stop=True)

            if FP8_Q:
                rx8_3d = RX8[:, :].rearrange("p (two n) -> p two n", two=2)
                wq8_3d = wq8[:, :].rearrange("p (two m) -> p two m", two=2)

            # --- software-pipelined GRU ---------------------------------
            # slot s: head(A=s): rz matmuls + fused r*h (custom DVE)
            #         mid (B=s-1): q matmul (into A's dead r-half of prz),
            #                      sigmoid(z), tanh(q)
            #         tail(C=s-2): blend d/e/h' on DVE+Pool
            # This keeps every in-order engine queue stall-free.
            state = {}  # slot -> dict(prz=, Z=, QT=, grp=)

            def emit_head(s):
                it, grp = slots[s]
                g0 = grp[0]
                W = len(grp) * F
                cols = slice(g0 * F, g0 * F + W)
                prz = przp.tile([128, 4 * F], dt.float32, tag="a")
                for j, g in enumerate(grp):      # r matmuls first
                    gc = slice(g * F, (g + 1) * F)
                    acc_mm(prz[:, j * F:(j + 1) * F],
                           WRH, WRX, Gb[:, gc], Xb[:, gc], 128)
                for j, g in enumerate(grp):
                    gc = slice(g * F, (g + 1) * F)
                    acc_mm(prz[:, 2 * F + j * F:2 * F + (j + 1) * F],
                           WZH, WZX, Gb[:, gc], Xb[:, gc], 128)
                if FP8_Q:
                    rh_out = RX8[:, cols]
                else:
                    rh_t = rhpool.tile([128, 2 * F], dt.bfloat16)
                    rh_out = rh_t[:, 0:W]
                nc.vector._custom_dve(
                    SIGMUL, out=rh_out, in0=prz[:, 0:W],
                    in1=Gb[:, cols], s0=SIG_C0, s1=SIG_C1, imm2=0.5)
                state[s] = dict(grp=grp, prz=prz, rh=rh_out, W=W)

            def emit_mid(s):
                st = state[s]
                grp, prz, W = st["grp"], st["prz"], st["W"]
                for j, g in enumerate(grp):
                    if FP8_Q:
                        nc.tensor.matmul(
                            out=prz[:, j * F:(j + 1) * F],
                            lhsT=wq8_3d,
                            rhs=rx8_3d[:, :, g * F:(g + 1) * F],
                            start=True, stop=True,
                            perf_mode=mybir.MatmulPerfMode.DoubleRow)
                    else:
                        acc_mm(prz[:, j * F:(j + 1) * F], WQH, WQX,
                               st["rh"][:, j * F:(j + 1) * F],
                               Xb[:, g * F:(g + 1) * F], 128)
                Z = zpool.tile([128, 2 * F], dt.bfloat16)
                nc.scalar.activation(
                    out=Z[:, 0:W], in_=prz[:, 2 * F:2 * F + W],
                    func=mybir.ActivationFunctionType.Sigmoid)
                QT = qtpool.tile([128, 2 * F], dt.bfloat16)
                nc.scalar.activation(
                    out=QT[:, 0:W], in_=prz[:, 0:W],
                    func=mybir.ActivationFunctionType.Tanh)
                st["Z"], st["QT"] = Z, QT

            def emit_de(s):
                st = state[s]
                grp, Z, QT = st["grp"], st["Z"], st["QT"]
                st["E"] = []
                for j, g in enumerate(grp):
                    gc = slice(g * F, (g + 1) * F)
                    jc = slice(j * F, (j + 1) * F)
                    D = dpool.tile([128, F], dt.bfloat16)
                    if _pool_d(g):
                        nc.gpsimd.tensor_tensor(
                            out=D, in0=QT[:, jc], in1=Gb[:, gc],
                            op=mybir.AluOpType.subtract)
                    else:
                        nc.vector.tensor_tensor(
                            out=D, in0=QT[:, jc], in1=Gb[:, gc],
                            op=mybir.AluOpType.subtract)
                    E = epool.tile([128, F], dt.bfloat16)
                    if _pool_e(g):
                        nc.gpsimd.tensor_mul(E, Z[:, jc], D)
                    else:
                        nc.vector.tensor_mul(E, Z[:, jc], D)
                    st["E"].append(E)

            def emit_hupd(s):
                st = state.pop(s)
                grp = st["grp"]
                for j, g in enumerate(grp):
                    gc = slice(g * F, (g + 1) * F)
                    E = st["E"][j]
                    if _pool_h(g):
                        nc.gpsimd.tensor_add(Gb[:, gc], Gb[:, gc], E)
                    else:
                        nc.vector.tensor_add(Gb[:, gc], Gb[:, gc], E)

            for _ in range(REPEATS):
                slots = [(it, grp) for it in range(NUM_ITERS)
                         for grp in GROUPS]
                for s in range(len(slots) + 3):
                    if s >= 3:
                        emit_hupd(s - 3)
                    if s >= 2 and s - 2 < len(slots):
                        emit_de(s - 2)
                    if s >= 1 and s - 1 < len(slots):
                        emit_mid(s - 1)
                    if s < len(slots):
                        emit_head(s)

                # decoder (pipelined, 2 groups = 4 pairs per round):
                # hmid = gelu(Wd1 @ [h; x]), flow = Wd2 @ hmid
                ROUNDS = [GROUPS[i:i + 2] for i in range(0, len(GROUPS), 2)]
                dstate = {}

                def dec_head(di):
                    pd = przp.tile([128, 2 * F], dt.float32, tag="a")
                    pairs = []
                    for hi, grp in enumerate(ROUNDS[di]):
                        for j, g in enumerate(grp):
                            gc = slice(g * F, (g + 1) * F)
                            acc_mm(pd[64 * j:64 * (j + 1),
                                      hi * F:(hi + 1) * F], WD1H, WD1X,
                                   Gb[:, gc], Xb[:, gc], 64)
                        pairs.append(grp)
                    dstate[di] = (pd, pairs)

                def dec_mid(di):
                    pd, pairs = dstate[di]
                    hm = hmpool.tile([128, 2 * F], dt.bfloat16)
                    for hi, grp in enumerate(pairs):
                        n = len(grp)
                        nc.scalar.activation(
                            out=hm[0:64 * n, hi * F:(hi + 1) * F],
                            in_=pd[0:64 * n, hi * F:(hi + 1) * F],
                            func=GELU_FUNC)
                    dstate[di] = (pd, pairs, hm)

                def dec_tail(di):
                    pd, pairs, hm = dstate.pop(di)
                    fl = flpool.tile([16, 2 * F], dt.float32)
                    for hi, grp in enumerate(pairs):
                        n = len(grp)
                        pf = pd[0:16, hi * F:(hi + 1) * F]
                        nc.tensor.matmul(
                            out=pf[0:8 * n, :],
                            lhsT=wts[0:64 * n, WD2Q:WD2Q + 8 * n],
                            rhs=hm[0:64 * n, hi * F:(hi + 1) * F],
                            start=True, stop=True)
                        nc.vector.tensor_copy(
                            out=fl[0:8 * n, hi * F:(hi + 1) * F],
                            in_=pf[0:8 * n, :])
                        for j, g in enumerate(grp):
                            nc.sync.dma_start(
                                out=flow_d[:, g * F:(g + 1) * F],
                                in_=fl[8 * j:8 * j + 8,
                                       hi * F:(hi + 1) * F])

                for di in range(len(ROUNDS) + 2):
                    if di >= 2:
                        dec_tail(di - 2)
                    if di >= 1 and di - 1 < len(ROUNDS):
                        dec_mid(di - 1)
                    if di < len(ROUNDS):
                        dec_head(di)

    nc.finalize()
    return nc


def _prep_host(before_feats, after_feats, point_feats, coords):
    """Build per-core input maps."""
    bf = np.asarray(before_feats)
    af = np.asarray(after_feats)
    pf = np.asarray(point_feats)
    cd = np.asarray(coords)
    assert cd.max() < GRID_SIDE and cd.min() >= 0, "coords out of 32^3 corner"

    # per-batch grid corner: [64, NCELL] f32
    grids = []
    for b in range(B):
        sub_b = bf[b, :, :GRID_SIDE, :GRID_SIDE, :GRID_SIDE]
        sub_a = af[b, :, :GRID_SIDE, :GRID_SIDE, :GRID_SIDE]
        grids.append(np.concatenate([sub_b, sub_a], axis=0)
                     .reshape(HID, NCELL))

    flat = ((cd[..., 0].astype(np.int64) * GRID_SIDE + cd[..., 1])
            * GRID_SIDE + cd[..., 2])               # [B, N]

    in_maps = []
    for core in range(NCORES):
        b, q = divmod(core, 4)
        sl = slice(q * NP_CORE, (q + 1) * NP_CORE)

        h0 = np.zeros((HID, NP_PAD), dtype=BF16)
        h0[:, :NP_CORE] = grids[b].take(flat[b, sl], axis=1).astype(BF16)
        # h0p[0:64] = h of A-halves (first 512 of each 1024), h0p[64:128] = B
        h0p = np.empty((128, NCOL), dtype=BF16)
        h03 = h0.reshape(HID, NPAIR, 2 * F)
        h0p[0:64] = h03[:, :, :F].reshape(HID, NCOL)
        h0p[64:128] = h03[:, :, F:].reshape(HID, NCOL)

        xt = np.zeros((PFEAT, NP_PAD), dtype=BF16)
        xt[:, :NP_CORE] = pf[b, sl].T.astype(BF16)
        xp = np.empty((128, NCOL), dtype=BF16)
        xt3 = xt.reshape(PFEAT, NPAIR, 2 * F)
        xp[0:64] = xt3[:, :, :F].reshape(PFEAT, NCOL)
        xp[64:128] = xt3[:, :, F:].reshape(PFEAT, NCOL)

        m = {
            "h0p": np.ascontiguousarray(h0p),
            "xp": np.ascontiguousarray(xp),
            "wts": _CACHED["wts"],
        }
        if FP8_Q:
            m["xp8"] = np.ascontiguousarray(xp.astype(F8E4))
            m["wq8"] = _CACHED["wq8"]
        in_maps.append(m)
    return in_maps


def _pack_weights(Wz, Wr, Wq, Wd1, Wd2):
    """Block-diagonal lhsT layouts: rows 0-63 (A state) feed the first
    half of the output columns, rows 64-127 (B) the second half."""
    w = np.zeros((128, WCOLS), dtype=BF16)
    Wzb = np.asarray(Wz).astype(np.float32)
    Wrb = np.asarray(Wr).astype(np.float32) / SIG_K   # pre-scale for SIGMUL
    Wqb = np.asarray(Wq).astype(np.float32)
    Wd1b = np.asarray(Wd1).astype(BF16)
    Wd2b = np.asarray(Wd2).astype(BF16)

    def blockdiag(col, wt):  # wt: lhsT block [64, m]
        m = wt.shape[1]
        w[0:64, col:col + m] = wt
        w[64:128, col + m:col + 2 * m] = wt

    blockdiag(WRH, Wrb[:, :HID].T.astype(BF16))
    blockdiag(WZH, Wzb[:, :HID].T.astype(BF16))
    blockdiag(WQH, Wqb[:, :HID].T.astype(BF16))
    blockdiag(WRX, Wrb[:, HID:].T.astype(BF16))
    blockdiag(WZX, Wzb[:, HID:].T.astype(BF16))
    blockdiag(WQX, Wqb[:, HID:].T.astype(BF16))
    blockdiag(WD1H, Wd1b[:, :HID].T)
    blockdiag(WD1X, Wd1b[:, HID:].T)
    # WD2Q: [128, 16]: 4 x 32-row blocks -> 4-col blocks (A/B of 2 pairs)
    for blk in range(4):
        w[32 * blk:32 * (blk + 1), WD2Q + 4 * blk:WD2Q + 4 * blk + 3] = \
            Wd2b.T

    # fp8 q weights, stacked for DoubleRow: cols 0:128 = Wq_h, 128:256 = Wq_x
    wq8 = np.zeros((128, 256), dtype=F8E4)
    wq8[0:64, 0:64] = Wqb[:, :HID].T.astype(F8E4)
    wq8[64:128, 64:128] = Wqb[:, :HID].T.astype(F8E4)
    wq8[0:64, 128:192] = Wqb[:, HID:].T.astype(F8E4)
    wq8[64:128, 192:256] = Wqb[:, HID:].T.astype(F8E4)
    return np.ascontiguousarray(w), np.ascontiguousarray(wq8)


def kernel(before_feats, after_feats, point_feats, coords,
           Wz, bz, Wr, br, Wq, bq, Wd1, bd1, Wd2, bd2):
    for bias in (bz, br, bq, bd1):
        assert np.abs(np.asarray(bias)).max() == 0.0, "nonzero bias unsupported"

    if "nc" not in _CACHED:
        _CACHED["nc"] = _build_program()
    _CACHED["wts"], _CACHED["wq8"] = _pack_weights(Wz, Wr, Wq, Wd1, Wd2)

    in_maps = _prep_host(before_feats, after_feats, point_feats, coords)
    res = run_bass_kernel_spmd(_CACHED["nc"], in_maps, list(range(NCORES)))
    _CACHED["last_exec_time_ns"] = res.exec_time_ns
    _CACHED["last_mean_exec_time_ns"] = res.mean_exec_time_ns

    out = np.empty((B, N, 3), dtype=np.float32)
    bd2v = np.asarray(bd2).astype(np.float32).reshape(1, 3)
    for core in range(NCORES):
        b, q = divmod(core, 4)
        fl = res.results[core]["flow"]          # [8, NCOL]
        fl3 = fl.reshape(8, NPAIR, F)
        per_pt = np.empty((3, NP_PAD), dtype=np.float32)
        pp = per_pt.reshape(3, NPAIR, 2 * F)
        pp[:, :, :F] = fl3[0:3]
        pp[:, :, F:] = fl3[4:7]
        out[b, q * NP_CORE:(q + 1) * NP_CORE, :] = \
            per_pt[:, :NP_CORE].T + bd2v
    return out
